# revision 4
# baseline (speedup 1.0000x reference)
"""Trainium2 Bass kernel for nn_AELoss (segment_reduce push/pull loss).

Strategy (data-parallel over batch rows, 8 NeuronCores):
  Per row (131072 elements, 129 segment ids):
  Phase 1 — per-bin count/sum/sumsq histogram via factored one-hot matmul:
    bin k = 16*c + f with c = g>>4 (9 coarse), f = g&15 (16 fine).
    DVE builds bf16 mask slabs: u = [d(c=j), x*d(c=j), x2*d(c=j)] (27 cols),
    v = [d(f=m)] (16 cols). TensorE contracts 128 elements per matmul:
    psum[27,16] += u_chunk.T @ v_chunk accumulated over all chunks.
  Phase 2 — per-row losses from the [27,16] stats (pull via algebraic
    identity sq = ssq - sum^2/cnt; push via KxK exp(-(mi-mj)^2) with
    invalid bins pushed to a huge sentinel mean, corrected in closed form).
"""
import functools
import numpy as np

import concourse.bacc as bacc
import concourse.bass as bass
import concourse.mybir as mybir
from concourse.bass_utils import run_bass_kernel_spmd
from concourse.tile import TileContext

F32 = mybir.dt.float32
BF16 = mybir.dt.bfloat16
I32 = mybir.dt.int32

B, N = 128, 131072
NCORES = 8
ROWS = B // NCORES  # rows per core
P = 128
NCOARSE, NFINE = 9, 16
NBINS = NCOARSE * NFINE  # 144 logical bins (129 real; 15 structurally empty)
BIG = 30000.0
AOT = mybir.AluOpType


def build(rows=ROWS, n=N, tile_f=512, debug_stats=False):
    cols = n // P              # elements per partition per row
    ntiles = cols // tile_f    # tiles per row
    assert cols % tile_f == 0

    nc = bacc.Bacc("TRN2", target_bir_lowering=False)
    tags_ext = nc.declare_dram_parameter("tags", [rows, n], F32, isOutput=False)
    gt_ext = nc.declare_dram_parameter("gt_tags", [rows, n], I32, isOutput=False)
    out_ext = nc.declare_dram_parameter("out", [2, rows], F32, isOutput=True)
    if debug_stats:
        stats_dbg = nc.declare_dram_parameter(
            "stats_dbg", [rows, 27, NFINE], F32, isOutput=True
        )

    with TileContext(nc) as tc:
        with (
            tc.tile_pool(name="io", bufs=3) as io_pool,
            tc.tile_pool(name="slab", bufs=2) as slab_pool,
            tc.tile_pool(name="small", bufs=2) as small_pool,
            tc.tile_pool(name="psum", bufs=2, space="PSUM") as psum_pool,
            tc.tile_pool(name="psum2", bufs=1, space="PSUM") as psum2_pool,
            tc.tile_pool(name="ph2", bufs=1) as ph2_pool,
            tc.tile_pool(name="dram", bufs=1, space="DRAM") as dram_pool,
        ):
            stats_dram = dram_pool.tile([rows, 27, NFINE], F32)
            mean_dram = dram_pool.tile([rows, NBINS], F32)
            # ---------------- Phase 1: histograms ----------------
            for r in range(rows):
                psum = psum_pool.tile([27, NFINE], F32)
                x_row = tags_ext[r].rearrange("(p c) -> p c", p=P)
                g_row = gt_ext[r].rearrange("(p c) -> p c", p=P)
                for h in range(ntiles):
                    sl = slice(h * tile_f, (h + 1) * tile_f)
                    xt = io_pool.tile([P, tile_f], F32, tag="xt")
                    gt = io_pool.tile([P, tile_f], I32, tag="gt")
                    nc.sync.dma_start(out=xt[:], in_=x_row[:, sl])
                    nc.sync.dma_start(out=gt[:], in_=g_row[:, sl])

                    xb = io_pool.tile([P, tile_f], BF16, tag="xb")
                    x2 = io_pool.tile([P, tile_f], BF16, tag="x2")
                    fi = io_pool.tile([P, tile_f], I32, tag="fi")
                    di = io_pool.tile([P, tile_f], I32, tag="di")
                    fb = io_pool.tile([P, tile_f], BF16, tag="fb")
                    db = io_pool.tile([P, tile_f], BF16, tag="db")
                    # casts / x^2 on ScalarE
                    nc.scalar.copy(xb[:], xt[:])
                    nc.scalar.activation(
                        x2[:], xb[:], mybir.ActivationFunctionType.Square
                    )
                    # f = g & 15 ; d = g & ~15 = 16*c  (int32), then cast to bf16
                    nc.vector.tensor_scalar(fi[:], gt[:], 15, None, AOT.bitwise_and)
                    nc.vector.tensor_scalar(di[:], gt[:], ~15, None, AOT.bitwise_and)
                    nc.vector.tensor_copy(fb[:], fi[:])
                    nc.vector.tensor_copy(db[:], di[:])

                    u = slab_pool.tile([P, 27, tile_f], BF16, tag="u")
                    v = slab_pool.tile([P, NFINE, tile_f], BF16, tag="v")
                    for j in range(NCOARSE):
                        tj = 16.0 * j
                        nc.vector.tensor_scalar(
                            u[:, j, :], db[:], tj, None, AOT.is_equal
                        )
                        nc.vector.scalar_tensor_tensor(
                            u[:, 9 + j, :], db[:], tj, xb[:], AOT.is_equal, AOT.mult
                        )
                        nc.vector.scalar_tensor_tensor(
                            u[:, 18 + j, :], db[:], tj, x2[:], AOT.is_equal, AOT.mult
                        )
                    for m in range(NFINE):
                        nc.vector.tensor_scalar(
                            v[:, m, :], fb[:], float(m), None, AOT.is_equal
                        )
                    for c in range(tile_f):
                        nc.tensor.matmul(
                            psum[:, :],
                            u[:, :, c],
                            v[:, :, c],
                            start=(h == 0 and c == 0),
                            stop=(h == ntiles - 1 and c == tile_f - 1),
                        )
                st = small_pool.tile([27, NFINE], F32, tag="st")
                nc.vector.tensor_copy(st[:], psum[:])
                nc.sync.dma_start(out=stats_dram[r, :, :], in_=st[:])
                if debug_stats:
                    nc.sync.dma_start(out=stats_dbg[r], in_=st[:])

            # ---------------- Phase 2: per-row losses ----------------
            # Strips of bins on partitions: A = bins 0..127, B = bins 128..143.
            strips = [(0, P), (P, NBINS - P)]
            # per-strip tiles kept for the pairwise stage
            strip_tiles = []
            # psum accumulators across strips: [1, rows] each
            acc_pull = psum2_pool.tile([1, rows], F32)   # sum_k sq_k/safe_k
            acc_T = psum2_pool.tile([1, rows], F32)      # sum_k valid_k
            ones_col = ph2_pool.tile([P, 1], F32)
            nc.vector.memset(ones_col[:], 1.0)

            sd = stats_dram
            for si, (k0, pk) in enumerate(strips):
                cnt = ph2_pool.tile([pk, rows], F32, tag=f"cnt{si}")
                sm = ph2_pool.tile([pk, rows], F32, tag=f"sm{si}")
                sq = ph2_pool.tile([pk, rows], F32, tag=f"sq{si}")
                # bins k = 16*j + m ; slab index for stat t is t*9+j
                # view [rows, 9, 16] -> [(j m) r] then slice strip
                for t, dst in ((0, cnt), (1, sm), (2, sq)):
                    src_ap = sd[:, t * 9 : (t + 1) * 9, :].rearrange(
                        "r j m -> (j m) r"
                    )[k0 : k0 + pk, :]
                    nc.sync.dma_start(out=dst[:], in_=src_ap)
                valid = ph2_pool.tile([pk, rows], F32, tag=f"va{si}")
                nc.vector.tensor_scalar(valid[:], cnt[:], 0.5, None, AOT.is_ge)
                safe = ph2_pool.tile([pk, rows], F32, tag=f"sa{si}")
                nc.vector.tensor_scalar(safe[:], cnt[:], 1.0, None, AOT.max)
                recip = ph2_pool.tile([pk, rows], F32, tag=f"re{si}")
                nc.vector.reciprocal(recip[:], safe[:])
                mean = ph2_pool.tile([pk, rows], F32, tag=f"me{si}")
                nc.vector.tensor_mul(mean[:], sm[:], recip[:])
                # sq_resid = ssq - sum*mean  (== ssq - sum^2/cnt)
                t1 = ph2_pool.tile([pk, rows], F32, tag=f"t1{si}")
                nc.vector.tensor_mul(t1[:], sm[:], mean[:])
                nc.vector.tensor_sub(sq[:], sq[:], t1[:])
                nc.vector.tensor_mul(sq[:], sq[:], recip[:])
                # mean with invalid bins at BIG sentinel
                mbig = ph2_pool.tile([pk, rows], F32, tag=f"mb{si}")
                nc.vector.tensor_scalar(
                    mbig[:], valid[:], -BIG, BIG, AOT.mult, AOT.add
                )
                nc.vector.tensor_add(mbig[:], mbig[:], mean[:])
                nc.sync.dma_start(
                    out=mean_dram[:, k0 : k0 + pk].rearrange("r k -> k r"),
                    in_=mbig[:],
                )
                # accumulate pull terms and T across strips via ones-matmul
                nc.tensor.matmul(
                    acc_pull[:, :],
                    ones_col[:pk, :],
                    sq[:],
                    start=(si == 0),
                    stop=(si == len(strips) - 1),
                )
                nc.tensor.matmul(
                    acc_T[:, :],
                    ones_col[:pk, :],
                    valid[:],
                    start=(si == 0),
                    stop=(si == len(strips) - 1),
                )
                strip_tiles.append((pk, mbig))

            # Broadcast all means [rows*NBINS] to every partition.
            flat = mean_dram[:, :].rearrange("r k -> (r k)")
            mb_all = ph2_pool.tile([P, rows * NBINS], F32, tag="mball")
            nc.sync.dma_start(out=mb_all[:], in_=flat.partition_broadcast(P))

            # Pairwise exp(-(mi-mj)^2) summed over j for each (i, r).
            acc_push = psum2_pool.tile([1, rows], F32)
            for si, (pk, mbig) in enumerate(strip_tiles):
                diff = ph2_pool.tile([pk, rows * NBINS], F32, tag=f"df{si}")
                mi_b = mbig[:].unsqueeze(2).to_broadcast([pk, rows, NBINS])
                nc.vector.tensor_sub(
                    diff[:].rearrange("p (r k) -> p r k", k=NBINS),
                    mb_all[:pk, :].rearrange("p (r k) -> p r k", k=NBINS),
                    mi_b,
                )
                nc.vector.tensor_mul(diff[:], diff[:], diff[:])
                pexp = ph2_pool.tile([pk, rows * NBINS], F32, tag=f"pe{si}")
                nc.scalar.activation(
                    pexp[:], diff[:], mybir.ActivationFunctionType.Exp, scale=-1.0
                )
                psum_red = ph2_pool.tile([pk, rows], F32, tag=f"pr{si}")
                nc.vector.tensor_reduce(
                    psum_red[:],
                    pexp[:].rearrange("p (r k) -> p r k", k=NBINS),
                    mybir.AxisListType.X,
                    AOT.add,
                )
                nc.tensor.matmul(
                    acc_push[:, :],
                    ones_col[:pk, :],
                    psum_red[:],
                    start=(si == 0),
                    stop=(si == len(strip_tiles) - 1),
                )

            # ---------------- Final scalar assembly ----------------
            Tv = ph2_pool.tile([1, rows], F32, tag="Tv")
            pullv = ph2_pool.tile([1, rows], F32, tag="pullv")
            pushv = ph2_pool.tile([1, rows], F32, tag="pushv")
            nc.vector.tensor_copy(Tv[:], acc_T[:])
            nc.vector.tensor_copy(pullv[:], acc_pull[:])
            nc.vector.tensor_copy(pushv[:], acc_push[:])

            w = ph2_pool.tile([1, rows], F32, tag="w")
            w2 = ph2_pool.tile([1, rows], F32, tag="w2")
            res_push = ph2_pool.tile([1, rows], F32, tag="res_push")
            res_pull = ph2_pool.tile([1, rows], F32, tag="res_pull")
            # pull_loss = pullv / max(T,1) * (T > 0)
            nc.vector.tensor_scalar(w[:], Tv[:], 1.0, None, AOT.max)
            nc.vector.reciprocal(w[:], w[:])
            nc.vector.tensor_mul(w[:], w[:], pullv[:])
            nc.vector.tensor_scalar(w2[:], Tv[:], 0.5, None, AOT.is_ge)
            nc.vector.tensor_mul(res_pull[:], w[:], w2[:])
            # push_loss = (pushv - (NBINS-T)^2 - T) / max((T-1)*T, 1) * 0.5 * (T>1)
            nc.vector.tensor_scalar(w[:], Tv[:], -1.0, float(NBINS), AOT.mult, AOT.add)
            nc.vector.tensor_mul(w[:], w[:], w[:])  # (NBINS-T)^2
            nc.vector.tensor_sub(pushv[:], pushv[:], w[:])
            nc.vector.tensor_sub(pushv[:], pushv[:], Tv[:])
            nc.vector.tensor_scalar(w[:], Tv[:], -1.0, None, AOT.add)  # T-1
            nc.vector.tensor_mul(w[:], w[:], Tv[:])
            nc.vector.tensor_scalar(w[:], w[:], 1.0, None, AOT.max)
            nc.vector.reciprocal(w[:], w[:])
            nc.vector.tensor_mul(pushv[:], pushv[:], w[:])
            nc.vector.tensor_scalar(pushv[:], pushv[:], 0.5, None, AOT.mult)
            nc.vector.tensor_scalar(w2[:], Tv[:], 1.5, None, AOT.is_ge)
            nc.vector.tensor_mul(res_push[:], pushv[:], w2[:])
            nc.sync.dma_start(out=out_ext[0:1, :], in_=res_push[:])
            nc.sync.dma_start(out=out_ext[1:2, :], in_=res_pull[:])

    nc.compile()
    return nc


@functools.cache
def _built():
    return build()


def kernel(tags: np.ndarray, gt_tags: np.ndarray):
    nc = _built()
    tags = np.ascontiguousarray(tags, dtype=np.float32)
    gt = np.ascontiguousarray(gt_tags, dtype=np.int32)
    in_maps = [
        {
            "tags": tags[i * ROWS : (i + 1) * ROWS],
            "gt_tags": gt[i * ROWS : (i + 1) * ROWS],
        }
        for i in range(NCORES)
    ]
    res = run_bass_kernel_spmd(nc, in_maps, core_ids=list(range(NCORES)))
    push = np.concatenate([res.results[i]["out"][0] for i in range(NCORES)])
    pull = np.concatenate([res.results[i]["out"][1] for i in range(NCORES)])
    return push.astype(np.float32), pull.astype(np.float32)


# revision 11
# speedup vs baseline: 1.2114x; 1.2114x over previous
"""Trainium2 Bass kernel for nn_AELoss (segment_reduce push/pull loss).

Strategy (data-parallel over batch rows, 8 NeuronCores):
  Per row (131072 elements, 129 segment ids):
  Phase 1 — per-bin count/sum/sumsq histogram via factored one-hot matmul:
    bin k = 16*c + f with c = g>>4 (9 coarse), f = g&15 (16 fine).
    DVE builds bf16 mask slabs: u = [d(c=j), x*d(c=j), x2*d(c=j)] (27 cols,
    moving side), v = [d(f=m)] (16 cols, stationary side). TensorE contracts
    128 elements per matmul; chunks rotate over the 4 PE column strips
    (tile_position=(0,32q)) so 4 matmuls run concurrently, producing 4
    partial histograms psum[32q+m, (t,j)] that are strip-summed per row.
  Phase 2 — per-row losses from the [16,27] stats (pull via algebraic
    identity sq = ssq - sum^2/cnt; push via KxK exp(-(mi-mj)^2) with
    invalid bins pushed to a huge sentinel mean, corrected in closed form).
"""
import functools
import numpy as np

import concourse.bacc as bacc
import concourse.bass as bass
import concourse.mybir as mybir
from concourse.bass_utils import run_bass_kernel_spmd
from concourse.tile import TileContext

F32 = mybir.dt.float32
BF16 = mybir.dt.bfloat16
I32 = mybir.dt.int32

B, N = 128, 131072
NCORES = 8
ROWS = B // NCORES  # rows per core
P = 128
NCOARSE, NFINE = 9, 16
NBINS = NCOARSE * NFINE  # 144 logical bins (129 real; 15 structurally empty)
BIG = 30000.0
AOT = mybir.AluOpType
ACTF = mybir.ActivationFunctionType


def build(rows=ROWS, n=N, tile_f=512, debug_stats=False):
    cols = n // P              # chunks per row
    ntiles = cols // tile_f    # tiles per row
    assert cols % tile_f == 0

    nc = bacc.Bacc("TRN2", target_bir_lowering=False)
    tags_ext = nc.declare_dram_parameter("tags", [rows, n], F32, isOutput=False)
    gt_ext = nc.declare_dram_parameter("gt_tags", [rows, n], I32, isOutput=False)
    out_ext = nc.declare_dram_parameter("out", [2, rows], F32, isOutput=True)
    if debug_stats:
        stats_dbg = nc.declare_dram_parameter(
            "stats_dbg", [rows, NFINE, 27], F32, isOutput=True
        )

    with TileContext(nc) as tc:
        with (
            tc.tile_pool(name="io", bufs=3) as io_pool,
            tc.tile_pool(name="slab", bufs=2) as slab_pool,
            tc.tile_pool(name="small", bufs=2) as small_pool,
            tc.tile_pool(name="psum", bufs=2, space="PSUM") as psum_pool,
            tc.tile_pool(name="psum2", bufs=1, space="PSUM") as psum2_pool,
            tc.tile_pool(name="ph2", bufs=1) as ph2_pool,
            tc.tile_pool(name="dram", bufs=1, space="DRAM") as dram_pool,
        ):
            stats_dram = dram_pool.tile([rows, 27, NFINE], F32)
            mean_dram = dram_pool.tile([rows, NBINS], F32)
            # ---------------- Phase 1: histograms ----------------
            for r in range(rows):
                psum = psum_pool.tile([P, 27], F32)
                x_row = tags_ext[r].rearrange("(p c) -> p c", p=P)
                g_row = gt_ext[r].rearrange("(p c) -> p c", p=P)
                for h in range(ntiles):
                    sl = slice(h * tile_f, (h + 1) * tile_f)
                    xt = io_pool.tile([P, tile_f], F32, tag="xt")
                    gt = io_pool.tile([P, tile_f], I32, tag="gt")
                    nc.sync.dma_start(out=xt[:], in_=x_row[:, sl])
                    nc.sync.dma_start(out=gt[:], in_=g_row[:, sl])

                    xb = io_pool.tile([P, tile_f], BF16, tag="xb")
                    x2 = io_pool.tile([P, tile_f], BF16, tag="x2")
                    fi = io_pool.tile([P, tile_f], I32, tag="fi")
                    di = io_pool.tile([P, tile_f], I32, tag="di")
                    fb = io_pool.tile([P, tile_f], BF16, tag="fb")
                    db = io_pool.tile([P, tile_f], BF16, tag="db")
                    # casts / x^2 on ScalarE
                    nc.scalar.copy(xb[:], xt[:])
                    nc.scalar.activation(x2[:], xb[:], ACTF.Square)
                    # f = g & 15 ; d = g & ~15 = 16*c  (int32), cast to bf16
                    nc.vector.tensor_scalar(fi[:], gt[:], 15, None, AOT.bitwise_and)
                    nc.vector.tensor_scalar(di[:], gt[:], ~15, None, AOT.bitwise_and)
                    nc.vector.tensor_copy(fb[:], fi[:])
                    nc.vector.tensor_copy(db[:], di[:])

                    u = slab_pool.tile([P, 27, tile_f], BF16, tag="u")
                    v = slab_pool.tile([P, NFINE, tile_f], BF16, tag="v")
                    for j in range(NCOARSE):
                        tj = 16.0 * j
                        nc.vector.tensor_scalar(
                            u[:, j, :], db[:], tj, None, AOT.is_equal
                        )
                        nc.vector.tensor_mul(u[:, 9 + j, :], u[:, j, :], xb[:])
                        nc.vector.tensor_mul(u[:, 18 + j, :], u[:, j, :], x2[:])
                    for m in range(NFINE):
                        nc.vector.tensor_scalar(
                            v[:, m, :], fb[:], float(m), None, AOT.is_equal
                        )
                    for c in range(tile_f):
                        cg = h * tile_f + c
                        q = cg % 4
                        nc.tensor.matmul(
                            psum[32 * q : 32 * q + NFINE, :],
                            v[:, :, c],
                            u[:, :, c],
                            start=(cg < 4),
                            stop=(cg >= cols - 4),
                            tile_position=(0, 32 * q),
                        )
                # strip-sum the 4 partial histograms -> [16, 27]
                pc = small_pool.tile([P, 27], F32, tag="pc")
                nc.vector.tensor_copy(pc[:], psum[:])
                cps = [pc[0:NFINE, :]]
                for q in range(1, 4):
                    cq = small_pool.tile([NFINE, 27], F32, tag=f"cq{q}")
                    nc.vector.tensor_copy(cq[:], pc[32 * q : 32 * q + NFINE, :])
                    cps.append(cq[:])
                s01 = small_pool.tile([NFINE, 27], F32, tag="s01")
                s23 = small_pool.tile([NFINE, 27], F32, tag="s23")
                st = small_pool.tile([NFINE, 27], F32, tag="st")
                nc.vector.tensor_add(s01[:], cps[0], cps[1])
                nc.vector.tensor_add(s23[:], cps[2], cps[3])
                nc.vector.tensor_add(st[:], s01[:], s23[:])
                nc.sync.dma_start(
                    out=stats_dram[r, :, :].rearrange("tj m -> m tj"), in_=st[:]
                )
                if debug_stats:
                    nc.sync.dma_start(out=stats_dbg[r], in_=st[:])

            # ---------------- Phase 2: per-row losses ----------------
            # Strips of bins on partitions: A = bins 0..127, B = bins 128..143.
            strips = [(0, P), (P, NBINS - P)]
            strip_tiles = []
            acc_pull = psum2_pool.tile([1, rows], F32)   # sum_k sq_k/safe_k
            acc_T = psum2_pool.tile([1, rows], F32)      # sum_k valid_k
            ones_col = ph2_pool.tile([P, 1], F32)
            nc.vector.memset(ones_col[:], 1.0)
            ones_row = ph2_pool.tile([1, P], F32)
            nc.vector.memset(ones_row[:], 1.0)

            sd = stats_dram  # [rows, 27(t*9+j), 16(m)]
            for si, (k0, pk) in enumerate(strips):
                cnt = ph2_pool.tile([pk, rows], F32, tag=f"cnt{si}")
                sm = ph2_pool.tile([pk, rows], F32, tag=f"sm{si}")
                sq = ph2_pool.tile([pk, rows], F32, tag=f"sq{si}")
                # bin k = 16*j + m ; stat t lives at slab t*9+j
                for t, dst in ((0, cnt), (1, sm), (2, sq)):
                    src_ap = sd[:, t * 9 : (t + 1) * 9, :].rearrange(
                        "r j m -> (j m) r"
                    )[k0 : k0 + pk, :]
                    nc.sync.dma_start(out=dst[:], in_=src_ap)
                valid = ph2_pool.tile([pk, rows], F32, tag=f"va{si}")
                nc.vector.tensor_scalar(valid[:], cnt[:], 0.5, None, AOT.is_ge)
                safe = ph2_pool.tile([pk, rows], F32, tag=f"sa{si}")
                nc.vector.tensor_scalar(safe[:], cnt[:], 1.0, None, AOT.max)
                recip = ph2_pool.tile([pk, rows], F32, tag=f"re{si}")
                nc.vector.reciprocal(recip[:], safe[:])
                mean = ph2_pool.tile([pk, rows], F32, tag=f"me{si}")
                nc.vector.tensor_mul(mean[:], sm[:], recip[:])
                # sq_resid = ssq - sum*mean  (== ssq - sum^2/cnt)
                t1 = ph2_pool.tile([pk, rows], F32, tag=f"t1{si}")
                nc.vector.tensor_mul(t1[:], sm[:], mean[:])
                nc.vector.tensor_sub(sq[:], sq[:], t1[:])
                nc.vector.tensor_mul(sq[:], sq[:], recip[:])
                # mean with invalid bins at BIG sentinel
                mbig = ph2_pool.tile([pk, rows], F32, tag=f"mb{si}")
                nc.vector.tensor_scalar(
                    mbig[:], valid[:], -BIG, BIG, AOT.mult, AOT.add
                )
                nc.vector.tensor_add(mbig[:], mbig[:], mean[:])
                nc.sync.dma_start(
                    out=mean_dram[:, k0 : k0 + pk].rearrange("r k -> k r"),
                    in_=mbig[:],
                )
                nc.tensor.matmul(
                    acc_pull[:, :],
                    ones_col[:pk, :],
                    sq[:],
                    start=(si == 0),
                    stop=(si == len(strips) - 1),
                )
                nc.tensor.matmul(
                    acc_T[:, :],
                    ones_col[:pk, :],
                    valid[:],
                    start=(si == 0),
                    stop=(si == len(strips) - 1),
                )
                strip_tiles.append((pk, mbig))

            # Broadcast all means [rows*NBINS] to every partition via
            # K=1 ones-matmul in 512-wide psum chunks.
            mfl = ph2_pool.tile([1, rows * NBINS], F32, tag="mfl")
            nc.sync.dma_start(
                out=mfl[:], in_=mean_dram[:, :].rearrange("r k -> (r k)").unsqueeze(0)
            )
            mb_all = ph2_pool.tile([P, rows * NBINS], F32, tag="mball")
            nb = rows * NBINS
            for o in range(0, nb, 512):
                w_ = min(512, nb - o)
                pb = psum2_pool.tile([P, 512], F32, tag="pbb")
                nc.tensor.matmul(
                    pb[:, :w_],
                    ones_row[:, :],
                    mfl[:, o : o + w_],
                    start=True,
                    stop=True,
                )
                nc.vector.tensor_copy(mb_all[:, o : o + w_], pb[:, :w_])

            # Pairwise exp(-(mi-mj)^2) summed over j for each (i, r).
            acc_push = psum2_pool.tile([1, rows], F32)
            for si, (pk, mbig) in enumerate(strip_tiles):
                diff = ph2_pool.tile([pk, rows * NBINS], F32, tag=f"df{si}")
                mi_b = mbig[:].unsqueeze(2).to_broadcast([pk, rows, NBINS])
                nc.vector.tensor_sub(
                    diff[:].rearrange("p (r k) -> p r k", k=NBINS),
                    mb_all[:pk, :].rearrange("p (r k) -> p r k", k=NBINS),
                    mi_b,
                )
                nc.vector.tensor_mul(diff[:], diff[:], diff[:])
                pexp = ph2_pool.tile([pk, rows * NBINS], F32, tag=f"pe{si}")
                nc.scalar.activation(pexp[:], diff[:], ACTF.Exp, scale=-1.0)
                psum_red = ph2_pool.tile([pk, rows], F32, tag=f"pr{si}")
                nc.vector.tensor_reduce(
                    psum_red[:],
                    pexp[:].rearrange("p (r k) -> p r k", k=NBINS),
                    mybir.AxisListType.X,
                    AOT.add,
                )
                nc.tensor.matmul(
                    acc_push[:, :],
                    ones_col[:pk, :],
                    psum_red[:],
                    start=(si == 0),
                    stop=(si == len(strip_tiles) - 1),
                )

            # ---------------- Final scalar assembly ----------------
            Tv = ph2_pool.tile([1, rows], F32, tag="Tv")
            pullv = ph2_pool.tile([1, rows], F32, tag="pullv")
            pushv = ph2_pool.tile([1, rows], F32, tag="pushv")
            nc.vector.tensor_copy(Tv[:], acc_T[:])
            nc.vector.tensor_copy(pullv[:], acc_pull[:])
            nc.vector.tensor_copy(pushv[:], acc_push[:])

            w = ph2_pool.tile([1, rows], F32, tag="w")
            w2 = ph2_pool.tile([1, rows], F32, tag="w2")
            res_push = ph2_pool.tile([1, rows], F32, tag="res_push")
            res_pull = ph2_pool.tile([1, rows], F32, tag="res_pull")
            # pull_loss = pullv / max(T,1) * (T > 0)
            nc.vector.tensor_scalar(w[:], Tv[:], 1.0, None, AOT.max)
            nc.vector.reciprocal(w[:], w[:])
            nc.vector.tensor_mul(w[:], w[:], pullv[:])
            nc.vector.tensor_scalar(w2[:], Tv[:], 0.5, None, AOT.is_ge)
            nc.vector.tensor_mul(res_pull[:], w[:], w2[:])
            # push_loss = (pushv - (NBINS-T)^2 - T) / max((T-1)*T, 1) * .5 * (T>1)
            nc.vector.tensor_scalar(w[:], Tv[:], -1.0, float(NBINS), AOT.mult, AOT.add)
            nc.vector.tensor_mul(w[:], w[:], w[:])  # (NBINS-T)^2
            nc.vector.tensor_sub(pushv[:], pushv[:], w[:])
            nc.vector.tensor_sub(pushv[:], pushv[:], Tv[:])
            nc.vector.tensor_scalar(w[:], Tv[:], -1.0, None, AOT.add)  # T-1
            nc.vector.tensor_mul(w[:], w[:], Tv[:])
            nc.vector.tensor_scalar(w[:], w[:], 1.0, None, AOT.max)
            nc.vector.reciprocal(w[:], w[:])
            nc.vector.tensor_mul(pushv[:], pushv[:], w[:])
            nc.vector.tensor_scalar(pushv[:], pushv[:], 0.5, None, AOT.mult)
            nc.vector.tensor_scalar(w2[:], Tv[:], 1.5, None, AOT.is_ge)
            nc.vector.tensor_mul(res_push[:], pushv[:], w2[:])
            nc.sync.dma_start(out=out_ext[0:1, :], in_=res_push[:])
            nc.sync.dma_start(out=out_ext[1:2, :], in_=res_pull[:])

    nc.compile()
    return nc


@functools.cache
def _built():
    return build()


def kernel(tags: np.ndarray, gt_tags: np.ndarray):
    nc = _built()
    tags = np.ascontiguousarray(tags, dtype=np.float32)
    gt = np.ascontiguousarray(gt_tags, dtype=np.int32)
    in_maps = [
        {
            "tags": tags[i * ROWS : (i + 1) * ROWS],
            "gt_tags": gt[i * ROWS : (i + 1) * ROWS],
        }
        for i in range(NCORES)
    ]
    res = run_bass_kernel_spmd(nc, in_maps, core_ids=list(range(NCORES)))
    push = np.concatenate([res.results[i]["out"][0] for i in range(NCORES)])
    pull = np.concatenate([res.results[i]["out"][1] for i in range(NCORES)])
    return push.astype(np.float32), pull.astype(np.float32)
